# revision 14
# baseline (speedup 1.0000x reference)
"""Trainium2 Bass kernel for the AdeptPolylineEncoder (segment_reduce).

Strategy (data-parallel over 8 cores, 64 segments each, feat-major layout):
  - All matmuls bf16 (f32 PSUM accumulation).
  - One merged loop over segments keeps the TensorEngine dense (HAM warm).
  - Mask handled OFF the PE: GPSIMD broadcasts an additive {0,-1e9} mask row
    to 128 partitions; DVE tensor_tensor_reduce fuses (x + mask) with the
    segment max-reduction, reading matmul results straight from PSUM/SBUF.
  - pool = relu(max(x) + bias) (relu/+bias commute with max).
  - concat([h, pooled]) @ W3 == h @ W3[:128] + (pooled @ W3[128:] + b3),
    the latter applied as a per-segment bias during the h2 eviction.
  - Engine balance per segment: PE 15 matmuls, ACT all PSUM evictions,
    DVE all pools + small ops, GPSIMD mask broadcast, PSUM exactly 8 banks.
"""

import numpy as np
import ml_dtypes

import concourse.bass as bass
import concourse.tile as tile
from concourse import bacc, mybir
from concourse.bass_utils import run_bass_kernel_spmd

BF = ml_dtypes.bfloat16

B, T, P, C = 4, 128, 1024, 32
NCORES = 8
SEGS = (B * T) // NCORES      # 64 segments per core
SUP = P                       # supertile = one segment = 1024 points
NSUP = SEGS
PTS = SEGS * P                # 65536 points per core
NEG = -1.0e9

_f32 = mybir.dt.float32
_bf16 = mybir.dt.bfloat16
AF = mybir.ActivationFunctionType
ALU = mybir.AluOpType
AX = mybir.AxisListType


def build_nc():
    nc = bacc.Bacc("TRN2", target_bir_lowering=False, debug=False)

    xdram = nc.declare_dram_parameter("x", [NSUP, 64, 512], _bf16, isOutput=False)
    mdram = nc.declare_dram_parameter("maskc", [NSUP, SUP], _bf16, isOutput=False)
    w1blk = nc.declare_dram_parameter("w1blk", [64, 128], _bf16, isOutput=False)
    w2stk = nc.declare_dram_parameter("w2stk", [128, 128], _bf16, isOutput=False)
    w3a = nc.declare_dram_parameter("w3a", [128, 256], _bf16, isOutput=False)
    w3b = nc.declare_dram_parameter("w3b", [128, 256], _bf16, isOutput=False)
    w4c0 = nc.declare_dram_parameter("w4c0", [128, 256], _bf16, isOutput=False)
    w4c1 = nc.declare_dram_parameter("w4c1", [128, 256], _bf16, isOutput=False)
    wo1c0 = nc.declare_dram_parameter("wo1c0", [128, 256], _bf16, isOutput=False)
    wo1c1 = nc.declare_dram_parameter("wo1c1", [128, 256], _bf16, isOutput=False)
    wo2c0 = nc.declare_dram_parameter("wo2c0", [128, 256], _bf16, isOutput=False)
    wo2c1 = nc.declare_dram_parameter("wo2c1", [128, 256], _bf16, isOutput=False)
    bo2r = nc.declare_dram_parameter("bo2row", [1, 256], _bf16, isOutput=False)
    bdram = nc.declare_dram_parameter("biases", [128, 8], _f32, isOutput=False)
    outd = nc.declare_dram_parameter("out", [128, 2, SEGS], _f32, isOutput=True)

    with tile.TileContext(nc) as tc:
        with (
            tc.tile_pool(name="consts", bufs=1) as cp,
            tc.tile_pool(name="h1pool", bufs=1) as hp,
        ):
            def csb(dram, shape, dt):
                t = cp.tile(shape, dt, name="c_" + dram.name, tag="c_" + dram.name)
                nc.sync.dma_start(out=t, in_=dram.ap())
                return t

            w1_s = csb(w1blk, [64, 128], _bf16)
            w2_s = csb(w2stk, [128, 128], _bf16)
            w3a_s = csb(w3a, [128, 256], _bf16)
            w3b_s = csb(w3b, [128, 256], _bf16)
            w4c0_s = csb(w4c0, [128, 256], _bf16)
            w4c1_s = csb(w4c1, [128, 256], _bf16)
            wo1c0_s = csb(wo1c0, [128, 256], _bf16)
            wo1c1_s = csb(wo1c1, [128, 256], _bf16)
            wo2c0_s = csb(wo2c0, [128, 256], _bf16)
            wo2c1_s = csb(wo2c1, [128, 256], _bf16)
            bo2r_s = csb(bo2r, [1, 256], _bf16)
            bias_s = csb(bdram, [128, 8], _f32)
            maskV = csb(mdram, [64, SUP], _bf16)   # row = segment

            # validity per segment: max over points of maskc (0 valid / NEG none)
            vr = cp.tile([64, 1], _f32)
            nc.vector.reduce_max(vr, maskV, axis=AX.X)
            vrow = cp.tile([1, 64], _f32)
            nc.sync.dma_start(out=vrow, in_=vr)    # partition column -> row
            vaddbf = cp.tile([1, 64], _bf16)
            nc.vector.tensor_copy(vaddbf, vrow)
            vbit = cp.tile([1, 64], _bf16)
            nc.vector.tensor_scalar(
                vbit, vrow, scalar1=1.0e-9, scalar2=1.0, op0=ALU.mult, op1=ALU.add
            )
            ones_s = cp.tile([1, 128], _bf16)
            nc.vector.memset(ones_s, 1.0)

            h1m = hp.tile([128, PTS], _bf16)        # h1 (unmasked), rhs of W3
            pooled1 = cp.tile([128, SEGS], _f32)    # raw masked max of h1
            p1bf = cp.tile([128, SEGS], _bf16)      # relu(pooled1)
            segB = cp.tile([128, 2, SEGS], _f32)    # per-seg bias for h2 evict
            pool2r = cp.tile([128, 2, SEGS], _f32)  # raw masked max of psum3

            with (
                tc.tile_pool(name="wk", bufs=3) as wk,
                tc.tile_pool(name="pA", bufs=2, space="PSUM") as pA,
                tc.tile_pool(name="pB", bufs=1, space="PSUM") as pB,
                tc.tile_pool(name="p2", bufs=2, space="PSUM") as p2,
                tc.tile_pool(name="p3", bufs=2, space="PSUM") as p3,
            ):
                for s in range(NSUP):
                    o = s * SUP
                    xa = wk.tile([64, 512], _bf16, tag="xa", name=f"xa{s}")
                    nc.sync.dma_start(out=xa, in_=xdram.ap()[s])
                    mrow = wk.tile([1, SUP], _bf16, tag="mrow", name=f"mr{s}")
                    nc.sync.dma_start(out=mrow, in_=mdram.ap()[s : s + 1, :])
                    mb = wk.tile([128, SUP], _bf16, tag="mb", name=f"mb{s}")
                    nc.gpsimd.partition_broadcast(mb, mrow[0:1, :])

                    # stage 1: h0 = relu(X @ W1 + b1), two 512-pt tiles stacked
                    psA = pA.tile([128, 512], _f32, tag="psA", name=f"psA{s}")
                    nc.tensor.matmul(psA, w1_s, xa, start=True, stop=True)
                    h0 = wk.tile([128, 512], _bf16, tag="h0", name=f"h0{s}")
                    nc.vector.tensor_scalar(
                        h0, psA, scalar1=bias_s[:, 0:1], scalar2=0.0,
                        op0=ALU.add, op1=ALU.max,
                    )

                    # stage 2: h1 = relu(h0 @ W2 + b2); masked pool1 via TTR
                    psB = pB.tile([128, 1024], _f32, tag="psB", name=f"psB{s}")
                    for n in range(2):
                        sl = slice(512 * n, 512 * n + 512)
                        nc.tensor.matmul(
                            psB[:, sl], w2_s[64 * n : 64 * n + 64, :],
                            h0[64 * n : 64 * n + 64, :], start=True, stop=True,
                        )
                    nc.scalar.activation(
                        h1m[:, o : o + SUP], psB, AF.Relu, bias=bias_s[:, 1:2]
                    )
                    scr1 = wk.tile([128, SUP], _bf16, tag="scr1", name=f"sc{s}")
                    nc.vector.tensor_tensor(
                        out=scr1, in0=h1m[:, o : o + SUP], in1=mb, op=ALU.add
                    )
                    nc.vector.reduce_max(
                        pooled1[:, s : s + 1], scr1, axis=AX.X
                    )
                    nc.vector.tensor_scalar(
                        p1bf[:, s : s + 1], pooled1[:, s : s + 1],
                        scalar1=0.0, scalar2=0.0, op0=ALU.add, op1=ALU.max,
                    )

                    # segment bias: segB = W3b.T @ relu(pool1) + b3
                    psS = pA.tile([128, 512], _f32, tag="psA", name=f"psS{s}")
                    for h in range(2):
                        nc.tensor.matmul(
                            psS[:, h : h + 1], w3b_s[:, 128 * h : 128 * h + 128],
                            p1bf[:, s : s + 1], start=True, stop=True,
                        )
                    for h in range(2):
                        nc.vector.tensor_scalar(
                            segB[:, h, s : s + 1], psS[:, h : h + 1],
                            scalar1=bias_s[:, 2 + h : 3 + h], scalar2=None,
                            op0=ALU.add,
                        )

                    # stage 3: h2 = relu(h1 @ W3a + segB), slice-wise
                    h2 = []
                    for h in range(2):
                        t = wk.tile([128, 1024], _bf16, tag=f"h2{h}", name=f"h2{h}_{s}")
                        for n in range(2):
                            sl = slice(512 * n, 512 * n + 512)
                            ps2 = p2.tile([128, 512], _f32, tag="ps2",
                                          name=f"p2_{h}_{n}_{s}")
                            nc.tensor.matmul(
                                ps2, w3a_s[:, 128 * h : 128 * h + 128],
                                h1m[:, o + 512 * n : o + 512 * n + 512],
                                start=True, stop=True,
                            )
                            nc.scalar.activation(
                                t[:, sl], ps2, AF.Relu, bias=segB[:, h, s : s + 1]
                            )
                        h2.append(t)

                    # stage 4: psum3 = h2 @ W4 - 1e9*invalid; pool2 per slice
                    part = wk.tile([128, 2, 2], _f32, tag="part", name=f"pt{s}")
                    for h in range(2):
                        for n in range(2):
                            sl = slice(512 * n, 512 * n + 512)
                            ps3 = p3.tile([128, 512], _f32, tag="ps3",
                                          name=f"p3_{h}_{n}_{s}")
                            nc.tensor.matmul(
                                ps3, w4c0_s[:, 128 * h : 128 * h + 128],
                                h2[0][:, sl], start=True, stop=False,
                            )
                            nc.tensor.matmul(
                                ps3, w4c1_s[:, 128 * h : 128 * h + 128],
                                h2[1][:, sl], start=False, stop=False,
                            )
                            nc.tensor.matmul(
                                ps3, ones_s, mrow[0:1, sl],
                                start=False, stop=True,
                            )
                            nc.vector.reduce_max(
                                part[:, h, n : n + 1], ps3, axis=AX.X
                            )
                    nc.vector.tensor_tensor(
                        out=pool2r[:, :, s], in0=part[:, :, 0], in1=part[:, :, 1],
                        op=ALU.max,
                    )

            # ---- tail: out-MLP on [256 x SEGS] with validity gating ----
            with (
                tc.tile_pool(name="tail", bufs=1) as tl,
                tc.tile_pool(name="pT", bufs=1, space="PSUM") as pT,
            ):
                featbf = tl.tile([128, 2, SEGS], _bf16)
                for h in range(2):
                    nc.vector.tensor_scalar(
                        featbf[:, h, :], pool2r[:, h, :],
                        scalar1=bias_s[:, 4 + h : 5 + h], scalar2=0.0,
                        op0=ALU.add, op1=ALU.max,
                    )
                psg = pT.tile([128, 128], _f32, tag="psg")
                for h in range(2):
                    sl = slice(64 * h, 64 * h + 64)
                    hs = slice(128 * h, 128 * h + 128)
                    nc.tensor.matmul(psg[:, sl], wo1c0_s[:, hs], featbf[:, 0, :],
                                     start=True, stop=False)
                    nc.tensor.matmul(psg[:, sl], wo1c1_s[:, hs], featbf[:, 1, :],
                                     start=False, stop=False)
                    nc.tensor.matmul(psg[:, sl], ones_s, vaddbf,
                                     start=False, stop=True)
                gbf = tl.tile([128, 2, SEGS], _bf16)
                for h in range(2):
                    nc.scalar.activation(
                        gbf[:, h, :], psg[:, 64 * h : 64 * h + 64], AF.Relu,
                        bias=bias_s[:, 6 + h : 7 + h],
                    )
                pso = pT.tile([128, 128], _f32, tag="pso")
                for h in range(2):
                    sl = slice(64 * h, 64 * h + 64)
                    hs = slice(128 * h, 128 * h + 128)
                    nc.tensor.matmul(pso[:, sl], wo2c0_s[:, hs], gbf[:, 0, :],
                                     start=True, stop=False)
                    nc.tensor.matmul(pso[:, sl], wo2c1_s[:, hs], gbf[:, 1, :],
                                     start=False, stop=False)
                    nc.tensor.matmul(pso[:, sl], bo2r_s[0:1, hs], vbit,
                                     start=False, stop=True)
                outsb = tl.tile([128, 2, SEGS], _f32)
                for h in range(2):
                    nc.vector.tensor_copy(outsb[:, h, :], pso[:, 64 * h : 64 * h + 64])
                nc.sync.dma_start(out=outd.ap(), in_=outsb)

    nc.finalize()
    return nc


def make_in_maps(lidar_points, lidar_mask, W1, b1, W2, b2, W3, b3, W4, b4,
                 Wo1, bo1, Wo2, bo2):
    f32 = np.float32
    pts = np.asarray(lidar_points, f32).reshape(B * T, P, C)
    msk = np.asarray(lidar_mask).reshape(B * T, P)

    W1 = np.asarray(W1, f32); W2 = np.asarray(W2, f32)
    W3 = np.asarray(W3, f32); W4 = np.asarray(W4, f32)
    Wo1 = np.asarray(Wo1, f32); Wo2 = np.asarray(Wo2, f32)

    w1blk = np.zeros((64, 128), f32)
    w1blk[0:32, 0:64] = W1
    w1blk[32:64, 64:128] = W1
    w2stk = np.concatenate([W2, W2], axis=0)          # [128,128]
    biases = np.zeros((128, 8), f32)
    biases[:, 0] = np.concatenate([b1, b1])
    biases[:, 1] = b2
    biases[:, 2] = b3[0:128]; biases[:, 3] = b3[128:256]
    biases[:, 4] = b4[0:128]; biases[:, 5] = b4[128:256]
    biases[:, 6] = bo1[0:128]; biases[:, 7] = bo1[128:256]

    shared = dict(
        w1blk=w1blk.astype(BF), w2stk=w2stk.astype(BF),
        w3a=W3[0:128].astype(BF), w3b=W3[128:256].astype(BF),
        w4c0=W4[0:128].astype(BF), w4c1=W4[128:256].astype(BF),
        wo1c0=Wo1[0:128].astype(BF), wo1c1=Wo1[128:256].astype(BF),
        wo2c0=Wo2[0:128].astype(BF), wo2c1=Wo2[128:256].astype(BF),
        bo2row=np.asarray(bo2, f32).reshape(1, 256).astype(BF),
        biases=biases,
    )

    in_maps = []
    for c in range(NCORES):
        seg0 = c * SEGS
        pc = pts[seg0 : seg0 + SEGS]                   # (64, 1024, 32)
        xc = pc.reshape(NSUP, 2, 512, C).transpose(0, 1, 3, 2).reshape(NSUP, 64, 512)
        mc = np.where(msk[seg0 : seg0 + SEGS], 0.0, NEG).astype(f32)  # (64, 1024)
        in_maps.append(dict(
            x=np.ascontiguousarray(xc).astype(BF),
            maskc=mc.astype(BF),
            **shared,
        ))
    return in_maps


def unshard(results):
    outs = []
    for c in range(NCORES):
        o = np.asarray(results[c]["out"], np.float32)   # [128, 2, SEGS]
        outs.append(o.transpose(2, 1, 0).reshape(SEGS, 256))
    return np.concatenate(outs, axis=0).reshape(B, T, 256)


_NC_CACHE = None


def _get_nc():
    global _NC_CACHE
    if _NC_CACHE is None:
        _NC_CACHE = build_nc()
    return _NC_CACHE


def run(trace=False, **inputs):
    nc = _get_nc()
    in_maps = make_in_maps(**inputs)
    res = run_bass_kernel_spmd(nc, in_maps, core_ids=list(range(NCORES)),
                               trace=trace)
    return unshard(res.results), res


def kernel(**inputs) -> np.ndarray:
    out, _ = run(trace=False, **inputs)
    return out


# revision 15
# speedup vs baseline: 1.5106x; 1.5106x over previous
"""Trainium2 Bass kernel for the AdeptPolylineEncoder (segment_reduce).

Strategy (data-parallel over 8 cores, 64 segments each, feat-major layout):
  - All matmuls bf16 (f32 PSUM accumulation).
  - One merged loop over segments keeps the TensorEngine dense (HAM warm).
  - Mask handled OFF the PE: GPSIMD broadcasts an additive {0,-1e9} mask row
    to 128 partitions; DVE tensor_tensor_reduce fuses (x + mask) with the
    segment max-reduction, reading matmul results straight from PSUM/SBUF.
  - pool = relu(max(x) + bias) (relu/+bias commute with max).
  - concat([h, pooled]) @ W3 == h @ W3[:128] + (pooled @ W3[128:] + b3),
    the latter applied as a per-segment bias during the h2 eviction.
  - Engine balance per segment: PE 15 matmuls, ACT all PSUM evictions,
    DVE all pools + small ops, GPSIMD mask broadcast, PSUM exactly 8 banks.
"""

import numpy as np
import ml_dtypes

import concourse.bass as bass
import concourse.tile as tile
from concourse import bacc, mybir
from concourse.bass_utils import run_bass_kernel_spmd

BF = ml_dtypes.bfloat16

B, T, P, C = 4, 128, 1024, 32
NCORES = 8
SEGS = (B * T) // NCORES      # 64 segments per core
SUP = P                       # supertile = one segment = 1024 points
NSUP = SEGS
PTS = SEGS * P                # 65536 points per core
NEG = -1.0e9

_f32 = mybir.dt.float32
_bf16 = mybir.dt.bfloat16
AF = mybir.ActivationFunctionType
ALU = mybir.AluOpType
AX = mybir.AxisListType


def build_nc():
    nc = bacc.Bacc("TRN2", target_bir_lowering=False, debug=False)

    xdram = nc.declare_dram_parameter("x", [NSUP, 64, 512], _bf16, isOutput=False)
    mdram = nc.declare_dram_parameter("maskc", [NSUP, SUP], _bf16, isOutput=False)
    w1blk = nc.declare_dram_parameter("w1blk", [64, 128], _bf16, isOutput=False)
    w2stk = nc.declare_dram_parameter("w2stk", [128, 128], _bf16, isOutput=False)
    w3a = nc.declare_dram_parameter("w3a", [128, 256], _bf16, isOutput=False)
    w3b = nc.declare_dram_parameter("w3b", [128, 256], _bf16, isOutput=False)
    w4c0 = nc.declare_dram_parameter("w4c0", [128, 256], _bf16, isOutput=False)
    w4c1 = nc.declare_dram_parameter("w4c1", [128, 256], _bf16, isOutput=False)
    wo1c0 = nc.declare_dram_parameter("wo1c0", [128, 256], _bf16, isOutput=False)
    wo1c1 = nc.declare_dram_parameter("wo1c1", [128, 256], _bf16, isOutput=False)
    wo2c0 = nc.declare_dram_parameter("wo2c0", [128, 256], _bf16, isOutput=False)
    wo2c1 = nc.declare_dram_parameter("wo2c1", [128, 256], _bf16, isOutput=False)
    bo2r = nc.declare_dram_parameter("bo2row", [1, 256], _bf16, isOutput=False)
    bdram = nc.declare_dram_parameter("biases", [128, 8], _f32, isOutput=False)
    outd = nc.declare_dram_parameter("out", [128, 2, SEGS], _f32, isOutput=True)

    with tile.TileContext(nc) as tc:
        with (
            tc.tile_pool(name="consts", bufs=1) as cp,
            tc.tile_pool(name="h1pool", bufs=1) as hp,
        ):
            def csb(dram, shape, dt):
                t = cp.tile(shape, dt, name="c_" + dram.name, tag="c_" + dram.name)
                nc.sync.dma_start(out=t, in_=dram.ap())
                return t

            w1_s = csb(w1blk, [64, 128], _bf16)
            w2_s = csb(w2stk, [128, 128], _bf16)
            w3a_s = csb(w3a, [128, 256], _bf16)
            w3b_s = csb(w3b, [128, 256], _bf16)
            w4c0_s = csb(w4c0, [128, 256], _bf16)
            w4c1_s = csb(w4c1, [128, 256], _bf16)
            wo1c0_s = csb(wo1c0, [128, 256], _bf16)
            wo1c1_s = csb(wo1c1, [128, 256], _bf16)
            wo2c0_s = csb(wo2c0, [128, 256], _bf16)
            wo2c1_s = csb(wo2c1, [128, 256], _bf16)
            bo2r_s = csb(bo2r, [1, 256], _bf16)
            bias_s = csb(bdram, [128, 8], _f32)
            maskV = csb(mdram, [64, SUP], _bf16)   # row = segment

            # validity per segment: max over points of maskc (0 valid / NEG none)
            vr = cp.tile([64, 1], _f32)
            nc.vector.reduce_max(vr, maskV, axis=AX.X)
            vrow = cp.tile([1, 64], _f32)
            nc.sync.dma_start(out=vrow, in_=vr)    # partition column -> row
            vaddbf = cp.tile([1, 64], _bf16)
            nc.vector.tensor_copy(vaddbf, vrow)
            vbit = cp.tile([1, 64], _bf16)
            nc.vector.tensor_scalar(
                vbit, vrow, scalar1=1.0e-9, scalar2=1.0, op0=ALU.mult, op1=ALU.add
            )
            ones_s = cp.tile([1, 128], _bf16)
            nc.vector.memset(ones_s, 1.0)

            h1m = hp.tile([128, PTS], _bf16)        # h1 (unmasked), rhs of W3
            pooled1 = cp.tile([128, SEGS], _f32)    # raw masked max of h1
            p1bf = cp.tile([128, SEGS], _bf16)      # relu(pooled1)
            segB = cp.tile([128, 2, SEGS], _f32)    # per-seg bias for h2 evict
            pool2r = cp.tile([128, 2, SEGS], _f32)  # raw masked max of psum3

            # ---- pass 1: X -> h0 -> h1 + masked pool1 ----
            with (
                tc.tile_pool(name="wk", bufs=3) as wk,
                tc.tile_pool(name="pA", bufs=2, space="PSUM") as pA,
                tc.tile_pool(name="pB", bufs=3, space="PSUM") as pB,
            ):
                for s in range(NSUP):
                    o = s * SUP
                    xa = wk.tile([64, 512], _bf16, tag="xa", name=f"xa{s}")
                    nc.sync.dma_start(out=xa, in_=xdram.ap()[s])
                    mrow = wk.tile([1, SUP], _bf16, tag="mrow", name=f"mr{s}")
                    nc.sync.dma_start(out=mrow, in_=mdram.ap()[s : s + 1, :])

                    # stage 1: h0 = relu(X @ W1 + b1), two 512-pt tiles stacked
                    psA = pA.tile([128, 512], _f32, tag="psA", name=f"psA{s}")
                    nc.tensor.matmul(psA, w1_s, xa, start=True, stop=True)
                    h0 = wk.tile([128, 512], _bf16, tag="h0", name=f"h0{s}")
                    if s % 2 == 0:
                        nc.scalar.activation(h0, psA, AF.Relu, bias=bias_s[:, 0:1])
                    else:
                        nc.vector.tensor_scalar(
                            h0, psA, scalar1=bias_s[:, 0:1], scalar2=0.0,
                            op0=ALU.add, op1=ALU.max,
                        )

                    # stage 2: psumB = h0 @ W2 - 1e9*invalid
                    psB = pB.tile([128, 1024], _f32, tag="psB", name=f"psB{s}")
                    for n in range(2):
                        sl = slice(512 * n, 512 * n + 512)
                        nc.tensor.matmul(
                            psB[:, sl], w2_s[64 * n : 64 * n + 64, :],
                            h0[64 * n : 64 * n + 64, :], start=True, stop=False,
                        )
                        nc.tensor.matmul(
                            psB[:, sl], ones_s, mrow[0:1, sl],
                            start=False, stop=True,
                        )
                    nc.scalar.activation(
                        h1m[:, o : o + SUP], psB, AF.Relu, bias=bias_s[:, 1:2]
                    )
                    nc.vector.reduce_max(pooled1[:, s : s + 1], psB, axis=AX.X)

            # ---- segment bias (batched): segB = W3b.T @ relu(pool1+b2) + b3 ----
            nc.vector.tensor_scalar(
                p1bf, pooled1, scalar1=bias_s[:, 1:2], scalar2=0.0,
                op0=ALU.add, op1=ALU.max,
            )
            with tc.tile_pool(name="pS", bufs=1, space="PSUM") as pS:
                psS = pS.tile([128, 128], _f32)
                for h in range(2):
                    nc.tensor.matmul(
                        psS[:, 64 * h : 64 * h + 64],
                        w3b_s[:, 128 * h : 128 * h + 128], p1bf,
                        start=True, stop=True,
                    )
                for h in range(2):
                    nc.vector.tensor_scalar(
                        segB[:, h, :], psS[:, 64 * h : 64 * h + 64],
                        scalar1=bias_s[:, 2 + h : 3 + h], scalar2=None,
                        op0=ALU.add,
                    )

            # ---- pass 2: h1 -> h2 -> psum3(masked) + pool2 ----
            with (
                tc.tile_pool(name="wk2", bufs=3) as wk2,
                tc.tile_pool(name="p2", bufs=3, space="PSUM") as p2,
                tc.tile_pool(name="p3", bufs=4, space="PSUM") as p3,
            ):
                for s in range(NSUP):
                    o = s * SUP
                    mrow = wk2.tile([1, SUP], _bf16, tag="mrow2", name=f"mr2_{s}")
                    nc.sync.dma_start(out=mrow, in_=mdram.ap()[s : s + 1, :])

                    # stage 3: h2 = relu(h1 @ W3a + segB), slice-wise
                    h2 = []
                    for h in range(2):
                        t = wk2.tile([128, 1024], _bf16, tag=f"h2{h}", name=f"h2{h}_{s}")
                        for n in range(2):
                            sl = slice(512 * n, 512 * n + 512)
                            ps2 = p2.tile([128, 512], _f32, tag="ps2",
                                          name=f"p2_{h}_{n}_{s}")
                            nc.tensor.matmul(
                                ps2, w3a_s[:, 128 * h : 128 * h + 128],
                                h1m[:, o + 512 * n : o + 512 * n + 512],
                                start=True, stop=True,
                            )
                            nc.scalar.activation(
                                t[:, sl], ps2, AF.Relu, bias=segB[:, h, s : s + 1]
                            )
                        h2.append(t)

                    # stage 4: psum3 = h2 @ W4 - 1e9*invalid; pool2 per slice
                    part = wk2.tile([128, 2, 2], _f32, tag="part", name=f"pt{s}")
                    for h in range(2):
                        for n in range(2):
                            sl = slice(512 * n, 512 * n + 512)
                            ps3 = p3.tile([128, 512], _f32, tag="ps3",
                                          name=f"p3_{h}_{n}_{s}")
                            nc.tensor.matmul(
                                ps3, w4c0_s[:, 128 * h : 128 * h + 128],
                                h2[0][:, sl], start=True, stop=False,
                            )
                            nc.tensor.matmul(
                                ps3, w4c1_s[:, 128 * h : 128 * h + 128],
                                h2[1][:, sl], start=False, stop=False,
                            )
                            nc.tensor.matmul(
                                ps3, ones_s, mrow[0:1, sl],
                                start=False, stop=True,
                            )
                            nc.vector.reduce_max(
                                part[:, h, n : n + 1], ps3, axis=AX.X
                            )
                    nc.vector.tensor_tensor(
                        out=pool2r[:, :, s], in0=part[:, :, 0], in1=part[:, :, 1],
                        op=ALU.max,
                    )

            # ---- tail: out-MLP on [256 x SEGS] with validity gating ----
            with (
                tc.tile_pool(name="tail", bufs=1) as tl,
                tc.tile_pool(name="pT", bufs=1, space="PSUM") as pT,
            ):
                featbf = tl.tile([128, 2, SEGS], _bf16)
                for h in range(2):
                    nc.vector.tensor_scalar(
                        featbf[:, h, :], pool2r[:, h, :],
                        scalar1=bias_s[:, 4 + h : 5 + h], scalar2=0.0,
                        op0=ALU.add, op1=ALU.max,
                    )
                psg = pT.tile([128, 128], _f32, tag="psg")
                for h in range(2):
                    sl = slice(64 * h, 64 * h + 64)
                    hs = slice(128 * h, 128 * h + 128)
                    nc.tensor.matmul(psg[:, sl], wo1c0_s[:, hs], featbf[:, 0, :],
                                     start=True, stop=False)
                    nc.tensor.matmul(psg[:, sl], wo1c1_s[:, hs], featbf[:, 1, :],
                                     start=False, stop=False)
                    nc.tensor.matmul(psg[:, sl], ones_s, vaddbf,
                                     start=False, stop=True)
                gbf = tl.tile([128, 2, SEGS], _bf16)
                for h in range(2):
                    nc.scalar.activation(
                        gbf[:, h, :], psg[:, 64 * h : 64 * h + 64], AF.Relu,
                        bias=bias_s[:, 6 + h : 7 + h],
                    )
                pso = pT.tile([128, 128], _f32, tag="pso")
                for h in range(2):
                    sl = slice(64 * h, 64 * h + 64)
                    hs = slice(128 * h, 128 * h + 128)
                    nc.tensor.matmul(pso[:, sl], wo2c0_s[:, hs], gbf[:, 0, :],
                                     start=True, stop=False)
                    nc.tensor.matmul(pso[:, sl], wo2c1_s[:, hs], gbf[:, 1, :],
                                     start=False, stop=False)
                    nc.tensor.matmul(pso[:, sl], bo2r_s[0:1, hs], vbit,
                                     start=False, stop=True)
                outsb = tl.tile([128, 2, SEGS], _f32)
                for h in range(2):
                    nc.vector.tensor_copy(outsb[:, h, :], pso[:, 64 * h : 64 * h + 64])
                nc.sync.dma_start(out=outd.ap(), in_=outsb)

    nc.finalize()
    return nc


def make_in_maps(lidar_points, lidar_mask, W1, b1, W2, b2, W3, b3, W4, b4,
                 Wo1, bo1, Wo2, bo2):
    f32 = np.float32
    pts = np.asarray(lidar_points, f32).reshape(B * T, P, C)
    msk = np.asarray(lidar_mask).reshape(B * T, P)

    W1 = np.asarray(W1, f32); W2 = np.asarray(W2, f32)
    W3 = np.asarray(W3, f32); W4 = np.asarray(W4, f32)
    Wo1 = np.asarray(Wo1, f32); Wo2 = np.asarray(Wo2, f32)

    w1blk = np.zeros((64, 128), f32)
    w1blk[0:32, 0:64] = W1
    w1blk[32:64, 64:128] = W1
    w2stk = np.concatenate([W2, W2], axis=0)          # [128,128]
    biases = np.zeros((128, 8), f32)
    biases[:, 0] = np.concatenate([b1, b1])
    biases[:, 1] = b2
    biases[:, 2] = b3[0:128]; biases[:, 3] = b3[128:256]
    biases[:, 4] = b4[0:128]; biases[:, 5] = b4[128:256]
    biases[:, 6] = bo1[0:128]; biases[:, 7] = bo1[128:256]

    shared = dict(
        w1blk=w1blk.astype(BF), w2stk=w2stk.astype(BF),
        w3a=W3[0:128].astype(BF), w3b=W3[128:256].astype(BF),
        w4c0=W4[0:128].astype(BF), w4c1=W4[128:256].astype(BF),
        wo1c0=Wo1[0:128].astype(BF), wo1c1=Wo1[128:256].astype(BF),
        wo2c0=Wo2[0:128].astype(BF), wo2c1=Wo2[128:256].astype(BF),
        bo2row=np.asarray(bo2, f32).reshape(1, 256).astype(BF),
        biases=biases,
    )

    in_maps = []
    for c in range(NCORES):
        seg0 = c * SEGS
        pc = pts[seg0 : seg0 + SEGS]                   # (64, 1024, 32)
        xc = pc.reshape(NSUP, 2, 512, C).transpose(0, 1, 3, 2).reshape(NSUP, 64, 512)
        mc = np.where(msk[seg0 : seg0 + SEGS], 0.0, NEG).astype(f32)  # (64, 1024)
        in_maps.append(dict(
            x=np.ascontiguousarray(xc).astype(BF),
            maskc=mc.astype(BF),
            **shared,
        ))
    return in_maps


def unshard(results):
    outs = []
    for c in range(NCORES):
        o = np.asarray(results[c]["out"], np.float32)   # [128, 2, SEGS]
        outs.append(o.transpose(2, 1, 0).reshape(SEGS, 256))
    return np.concatenate(outs, axis=0).reshape(B, T, 256)


_NC_CACHE = None


def _get_nc():
    global _NC_CACHE
    if _NC_CACHE is None:
        _NC_CACHE = build_nc()
    return _NC_CACHE


def run(trace=False, **inputs):
    nc = _get_nc()
    in_maps = make_in_maps(**inputs)
    res = run_bass_kernel_spmd(nc, in_maps, core_ids=list(range(NCORES)),
                               trace=trace)
    return unshard(res.results), res


def kernel(**inputs) -> np.ndarray:
    out, _ = run(trace=False, **inputs)
    return out


# revision 17
# speedup vs baseline: 1.6135x; 1.0681x over previous
"""Trainium2 Bass kernel for the AdeptPolylineEncoder (segment_reduce).

Strategy (data-parallel over 8 cores, 64 segments each, feat-major layout):
  - All matmuls bf16 (f32 PSUM accumulation).
  - One merged loop over segments keeps the TensorEngine dense (HAM warm).
  - Mask handled OFF the PE: GPSIMD broadcasts an additive {0,-1e9} mask row
    to 128 partitions; DVE tensor_tensor_reduce fuses (x + mask) with the
    segment max-reduction, reading matmul results straight from PSUM/SBUF.
  - pool = relu(max(x) + bias) (relu/+bias commute with max).
  - concat([h, pooled]) @ W3 == h @ W3[:128] + (pooled @ W3[128:] + b3),
    the latter applied as a per-segment bias during the h2 eviction.
  - Engine balance per segment: PE 15 matmuls, ACT all PSUM evictions,
    DVE all pools + small ops, GPSIMD mask broadcast, PSUM exactly 8 banks.
"""

import numpy as np
import ml_dtypes

import concourse.bass as bass
import concourse.tile as tile
from concourse import bacc, mybir
from concourse.bass_utils import run_bass_kernel_spmd

BF = ml_dtypes.bfloat16

B, T, P, C = 4, 128, 1024, 32
NCORES = 8
SEGS = (B * T) // NCORES      # 64 segments per core
SUP = P                       # supertile = one segment = 1024 points
NSUP = SEGS
PTS = SEGS * P                # 65536 points per core
NEG = -1.0e9

_f32 = mybir.dt.float32
_bf16 = mybir.dt.bfloat16
AF = mybir.ActivationFunctionType
ALU = mybir.AluOpType
AX = mybir.AxisListType


def build_nc():
    nc = bacc.Bacc("TRN2", target_bir_lowering=False, debug=False)

    xdram = nc.declare_dram_parameter("x", [NSUP, 64, 512], _bf16, isOutput=False)
    mdram = nc.declare_dram_parameter("maskc", [NSUP, SUP], _bf16, isOutput=False)
    w1blk = nc.declare_dram_parameter("w1blk", [64, 128], _bf16, isOutput=False)
    w2stk = nc.declare_dram_parameter("w2stk", [128, 128], _bf16, isOutput=False)
    w3a = nc.declare_dram_parameter("w3a", [128, 256], _bf16, isOutput=False)
    w3b = nc.declare_dram_parameter("w3b", [128, 256], _bf16, isOutput=False)
    w4c0 = nc.declare_dram_parameter("w4c0", [128, 256], _bf16, isOutput=False)
    w4c1 = nc.declare_dram_parameter("w4c1", [128, 256], _bf16, isOutput=False)
    wo1c0 = nc.declare_dram_parameter("wo1c0", [128, 256], _bf16, isOutput=False)
    wo1c1 = nc.declare_dram_parameter("wo1c1", [128, 256], _bf16, isOutput=False)
    wo2c0 = nc.declare_dram_parameter("wo2c0", [128, 256], _bf16, isOutput=False)
    wo2c1 = nc.declare_dram_parameter("wo2c1", [128, 256], _bf16, isOutput=False)
    bo2r = nc.declare_dram_parameter("bo2row", [1, 256], _bf16, isOutput=False)
    bdram = nc.declare_dram_parameter("biases", [128, 8], _f32, isOutput=False)
    outd = nc.declare_dram_parameter("out", [128, 2, SEGS], _f32, isOutput=True)

    with tile.TileContext(nc) as tc:
        with (
            tc.tile_pool(name="consts", bufs=1) as cp,
            tc.tile_pool(name="h1pool", bufs=1) as hp,
        ):
            def csb(dram, shape, dt):
                t = cp.tile(shape, dt, name="c_" + dram.name, tag="c_" + dram.name)
                nc.sync.dma_start(out=t, in_=dram.ap())
                return t

            w1_s = csb(w1blk, [64, 128], _bf16)
            w2_s = csb(w2stk, [128, 128], _bf16)
            w3a_s = csb(w3a, [128, 256], _bf16)
            w3b_s = csb(w3b, [128, 256], _bf16)
            w4c0_s = csb(w4c0, [128, 256], _bf16)
            w4c1_s = csb(w4c1, [128, 256], _bf16)
            wo1c0_s = csb(wo1c0, [128, 256], _bf16)
            wo1c1_s = csb(wo1c1, [128, 256], _bf16)
            wo2c0_s = csb(wo2c0, [128, 256], _bf16)
            wo2c1_s = csb(wo2c1, [128, 256], _bf16)
            bo2r_s = csb(bo2r, [1, 256], _bf16)
            bias_s = csb(bdram, [128, 8], _f32)
            maskV = csb(mdram, [64, SUP], _bf16)   # row = segment

            # validity per segment: max over points of maskc (0 valid / NEG none)
            vr = cp.tile([64, 1], _f32)
            nc.vector.reduce_max(vr, maskV, axis=AX.X)
            vrow = cp.tile([1, 64], _f32)
            nc.sync.dma_start(out=vrow, in_=vr)    # partition column -> row
            vaddbf = cp.tile([1, 64], _bf16)
            nc.vector.tensor_copy(vaddbf, vrow)
            vbit = cp.tile([1, 64], _bf16)
            nc.vector.tensor_scalar(
                vbit, vrow, scalar1=1.0e-9, scalar2=1.0, op0=ALU.mult, op1=ALU.add
            )
            ones_s = cp.tile([1, 128], _bf16)
            nc.vector.memset(ones_s, 1.0)

            h1m = hp.tile([128, PTS], _bf16)        # h1 (unmasked), rhs of W3
            pooled1 = cp.tile([128, SEGS], _f32)    # raw masked max of h1
            p1bf = cp.tile([128, SEGS], _bf16)      # relu(pooled1)
            segB = cp.tile([128, 2, SEGS], _f32)    # per-seg bias for h2 evict
            pool2r = cp.tile([128, 2, SEGS], _f32)  # raw masked max of psum3

            # ---- pass 1: X -> h0 -> h1 + masked pool1 (SW-pipelined) ----
            with (
                tc.tile_pool(name="wk", bufs=4) as wk,
                tc.tile_pool(name="pA", bufs=2, space="PSUM") as pA,
                tc.tile_pool(name="pB", bufs=3, space="PSUM") as pB,
            ):
                h0s = {}
                mrows = {}
                for s in range(NSUP + 1):
                    if s < NSUP:
                        xa = wk.tile([64, 512], _bf16, tag="xa", name=f"xa{s}")
                        nc.sync.dma_start(out=xa, in_=xdram.ap()[s])
                        mrow = wk.tile([1, SUP], _bf16, tag="mrow", name=f"mr{s}")
                        nc.sync.dma_start(out=mrow, in_=mdram.ap()[s : s + 1, :])
                        mrows[s] = mrow

                        # stage 1: h0 = relu(X @ W1 + b1), 2 point-tiles stacked
                        psA = pA.tile([128, 512], _f32, tag="psA", name=f"psA{s}")
                        nc.tensor.matmul(psA, w1_s, xa, start=True, stop=True)
                        h0 = wk.tile([128, 512], _bf16, tag="h0", name=f"h0{s}")
                        if s % 2 == 0:
                            nc.scalar.activation(h0, psA, AF.Relu,
                                                 bias=bias_s[:, 0:1])
                        else:
                            nc.vector.tensor_scalar(
                                h0, psA, scalar1=bias_s[:, 0:1], scalar2=0.0,
                                op0=ALU.add, op1=ALU.max,
                            )
                        h0s[s] = h0
                    if s >= 1:
                        p = s - 1
                        o = p * SUP
                        h0 = h0s.pop(p)
                        mrow = mrows.pop(p)
                        # stage 2: psumB = h0 @ W2 - 1e9*invalid
                        psB = pB.tile([128, 1024], _f32, tag="psB", name=f"psB{p}")
                        for n in range(2):
                            sl = slice(512 * n, 512 * n + 512)
                            nc.tensor.matmul(
                                psB[:, sl], w2_s[64 * n : 64 * n + 64, :],
                                h0[64 * n : 64 * n + 64, :], start=True, stop=False,
                            )
                            nc.tensor.matmul(
                                psB[:, sl], ones_s, mrow[0:1, sl],
                                start=False, stop=True,
                            )
                        nc.scalar.activation(
                            h1m[:, o : o + SUP], psB, AF.Relu, bias=bias_s[:, 1:2]
                        )
                        nc.vector.reduce_max(pooled1[:, p : p + 1], psB, axis=AX.X)

            # ---- segment bias (batched): segB = W3b.T @ relu(pool1+b2) + b3 ----
            nc.vector.tensor_scalar(
                p1bf, pooled1, scalar1=bias_s[:, 1:2], scalar2=0.0,
                op0=ALU.add, op1=ALU.max,
            )
            with tc.tile_pool(name="pS", bufs=1, space="PSUM") as pS:
                psS = pS.tile([128, 128], _f32)
                for h in range(2):
                    nc.tensor.matmul(
                        psS[:, 64 * h : 64 * h + 64],
                        w3b_s[:, 128 * h : 128 * h + 128], p1bf,
                        start=True, stop=True,
                    )
                for h in range(2):
                    nc.vector.tensor_scalar(
                        segB[:, h, :], psS[:, 64 * h : 64 * h + 64],
                        scalar1=bias_s[:, 2 + h : 3 + h], scalar2=None,
                        op0=ALU.add,
                    )

            # ---- pass 2: h1 -> h2 -> psum3(masked) + pool2 (SW-pipelined) ----
            with (
                tc.tile_pool(name="wk2", bufs=3) as wk2,
                tc.tile_pool(name="p2", bufs=3, space="PSUM") as p2,
                tc.tile_pool(name="p3", bufs=4, space="PSUM") as p3,
            ):
                h2s = {}
                mrows = {}
                for s in range(NSUP + 1):
                    if s < NSUP:
                        o = s * SUP
                        mrow = wk2.tile([1, SUP], _bf16, tag="mrow2",
                                        name=f"mr2_{s}")
                        nc.sync.dma_start(out=mrow, in_=mdram.ap()[s : s + 1, :])
                        mrows[s] = mrow

                        # stage 3: h2 = relu(h1 @ W3a + segB), slice-wise
                        h2 = []
                        for h in range(2):
                            t = wk2.tile([128, 1024], _bf16, tag=f"h2{h}",
                                         name=f"h2{h}_{s}")
                            for n in range(2):
                                sl = slice(512 * n, 512 * n + 512)
                                ps2 = p2.tile([128, 512], _f32, tag="ps2",
                                              name=f"p2_{h}_{n}_{s}")
                                nc.tensor.matmul(
                                    ps2, w3a_s[:, 128 * h : 128 * h + 128],
                                    h1m[:, o + 512 * n : o + 512 * n + 512],
                                    start=True, stop=True,
                                )
                                nc.scalar.activation(
                                    t[:, sl], ps2, AF.Relu,
                                    bias=segB[:, h, s : s + 1],
                                )
                            h2.append(t)
                        h2s[s] = h2
                    if s >= 1:
                        p = s - 1
                        h2 = h2s.pop(p)
                        mrow = mrows.pop(p)
                        # stage 4: psum3 = h2 @ W4 - 1e9*invalid; pool2/slice
                        part = wk2.tile([128, 2, 2], _f32, tag="part",
                                        name=f"pt{p}")
                        for h in range(2):
                            for n in range(2):
                                sl = slice(512 * n, 512 * n + 512)
                                ps3 = p3.tile([128, 512], _f32, tag="ps3",
                                              name=f"p3_{h}_{n}_{p}")
                                nc.tensor.matmul(
                                    ps3, w4c0_s[:, 128 * h : 128 * h + 128],
                                    h2[0][:, sl], start=True, stop=False,
                                )
                                nc.tensor.matmul(
                                    ps3, w4c1_s[:, 128 * h : 128 * h + 128],
                                    h2[1][:, sl], start=False, stop=False,
                                )
                                nc.tensor.matmul(
                                    ps3, ones_s, mrow[0:1, sl],
                                    start=False, stop=True,
                                )
                                nc.vector.reduce_max(
                                    part[:, h, n : n + 1], ps3, axis=AX.X
                                )
                        nc.vector.tensor_tensor(
                            out=pool2r[:, :, p], in0=part[:, :, 0],
                            in1=part[:, :, 1], op=ALU.max,
                        )

            # ---- tail: out-MLP on [256 x SEGS] with validity gating ----
            with (
                tc.tile_pool(name="tail", bufs=1) as tl,
                tc.tile_pool(name="pT", bufs=1, space="PSUM") as pT,
            ):
                featbf = tl.tile([128, 2, SEGS], _bf16)
                for h in range(2):
                    nc.vector.tensor_scalar(
                        featbf[:, h, :], pool2r[:, h, :],
                        scalar1=bias_s[:, 4 + h : 5 + h], scalar2=0.0,
                        op0=ALU.add, op1=ALU.max,
                    )
                psg = pT.tile([128, 128], _f32, tag="psg")
                for h in range(2):
                    sl = slice(64 * h, 64 * h + 64)
                    hs = slice(128 * h, 128 * h + 128)
                    nc.tensor.matmul(psg[:, sl], wo1c0_s[:, hs], featbf[:, 0, :],
                                     start=True, stop=False)
                    nc.tensor.matmul(psg[:, sl], wo1c1_s[:, hs], featbf[:, 1, :],
                                     start=False, stop=False)
                    nc.tensor.matmul(psg[:, sl], ones_s, vaddbf,
                                     start=False, stop=True)
                gbf = tl.tile([128, 2, SEGS], _bf16)
                for h in range(2):
                    nc.scalar.activation(
                        gbf[:, h, :], psg[:, 64 * h : 64 * h + 64], AF.Relu,
                        bias=bias_s[:, 6 + h : 7 + h],
                    )
                pso = pT.tile([128, 128], _f32, tag="pso")
                for h in range(2):
                    sl = slice(64 * h, 64 * h + 64)
                    hs = slice(128 * h, 128 * h + 128)
                    nc.tensor.matmul(pso[:, sl], wo2c0_s[:, hs], gbf[:, 0, :],
                                     start=True, stop=False)
                    nc.tensor.matmul(pso[:, sl], wo2c1_s[:, hs], gbf[:, 1, :],
                                     start=False, stop=False)
                    nc.tensor.matmul(pso[:, sl], bo2r_s[0:1, hs], vbit,
                                     start=False, stop=True)
                outsb = tl.tile([128, 2, SEGS], _f32)
                for h in range(2):
                    nc.vector.tensor_copy(outsb[:, h, :], pso[:, 64 * h : 64 * h + 64])
                nc.sync.dma_start(out=outd.ap(), in_=outsb)

    nc.finalize()
    return nc


def make_in_maps(lidar_points, lidar_mask, W1, b1, W2, b2, W3, b3, W4, b4,
                 Wo1, bo1, Wo2, bo2):
    f32 = np.float32
    pts = np.asarray(lidar_points, f32).reshape(B * T, P, C)
    msk = np.asarray(lidar_mask).reshape(B * T, P)

    W1 = np.asarray(W1, f32); W2 = np.asarray(W2, f32)
    W3 = np.asarray(W3, f32); W4 = np.asarray(W4, f32)
    Wo1 = np.asarray(Wo1, f32); Wo2 = np.asarray(Wo2, f32)

    w1blk = np.zeros((64, 128), f32)
    w1blk[0:32, 0:64] = W1
    w1blk[32:64, 64:128] = W1
    w2stk = np.concatenate([W2, W2], axis=0)          # [128,128]
    biases = np.zeros((128, 8), f32)
    biases[:, 0] = np.concatenate([b1, b1])
    biases[:, 1] = b2
    biases[:, 2] = b3[0:128]; biases[:, 3] = b3[128:256]
    biases[:, 4] = b4[0:128]; biases[:, 5] = b4[128:256]
    biases[:, 6] = bo1[0:128]; biases[:, 7] = bo1[128:256]

    shared = dict(
        w1blk=w1blk.astype(BF), w2stk=w2stk.astype(BF),
        w3a=W3[0:128].astype(BF), w3b=W3[128:256].astype(BF),
        w4c0=W4[0:128].astype(BF), w4c1=W4[128:256].astype(BF),
        wo1c0=Wo1[0:128].astype(BF), wo1c1=Wo1[128:256].astype(BF),
        wo2c0=Wo2[0:128].astype(BF), wo2c1=Wo2[128:256].astype(BF),
        bo2row=np.asarray(bo2, f32).reshape(1, 256).astype(BF),
        biases=biases,
    )

    in_maps = []
    for c in range(NCORES):
        seg0 = c * SEGS
        pc = pts[seg0 : seg0 + SEGS]                   # (64, 1024, 32)
        xc = pc.reshape(NSUP, 2, 512, C).transpose(0, 1, 3, 2).reshape(NSUP, 64, 512)
        mc = np.where(msk[seg0 : seg0 + SEGS], 0.0, NEG).astype(f32)  # (64, 1024)
        in_maps.append(dict(
            x=np.ascontiguousarray(xc).astype(BF),
            maskc=mc.astype(BF),
            **shared,
        ))
    return in_maps


def unshard(results):
    outs = []
    for c in range(NCORES):
        o = np.asarray(results[c]["out"], np.float32)   # [128, 2, SEGS]
        outs.append(o.transpose(2, 1, 0).reshape(SEGS, 256))
    return np.concatenate(outs, axis=0).reshape(B, T, 256)


_NC_CACHE = None


def _get_nc():
    global _NC_CACHE
    if _NC_CACHE is None:
        _NC_CACHE = build_nc()
    return _NC_CACHE


def run(trace=False, **inputs):
    nc = _get_nc()
    in_maps = make_in_maps(**inputs)
    res = run_bass_kernel_spmd(nc, in_maps, core_ids=list(range(NCORES)),
                               trace=trace)
    return unshard(res.results), res


def kernel(**inputs) -> np.ndarray:
    out, _ = run(trace=False, **inputs)
    return out
